# revision 3
# baseline (speedup 1.0000x reference)
"""Balanced top-2 MoE router on 8 TRN2 NeuronCores (token-parallel).

Reference computation:
    logits = hidden @ gate.T           [T=16384, E=64], H=4096
    probs  = softmax(logits); (w, i) = top2(probs); w /= (w.sum() + 1e-9)
    lb_loss / z_loss / expert_util from counts, P-mean and logsumexp stats.

Sharding: tokens split 8 ways (2048/core), gate replicated (sharding hint).

Host side packs both operands into an fp16 (hi, lo) pair representation
(hi = fp16(x), lo = fp16(x - hi)); the pair occupies the same 4 bytes per
element as the original fp32, so DMA volume is unchanged and the device
kernel reconstructs the full-precision product as
  (ghi + glo)^T @ (xhi + xlo)
with four fp16 matmul terms that run at 4x the fp32 matmul rate.

Per-core device kernel (per 512-token super-tile):
    - DMA hidden slice in natural [token, H] layout (contiguous, full BW)
    - PE transpose-mode matmuls turn [128t, 128h] blocks into [128h, 128t]
    - PSUM -> SBUF copies (split DVE/ACT) assemble xT [128h, 512t] tiles
    - 2 fp16 matmuls per contraction chunk: stationary [ghi|glo] [128h, 128],
      moving = hi/lo planes of xT via stride-2 fp16 APs -> PSUM [128, 512t];
      logits = top half + bottom half
    - PE transpose logits back to [128t, 64e]
    - softmax: ACT Exp with fused per-token bias (-max) + fused row-sum;
      top-2 via DVE InstMax/InstMaxIndex (ties resolve like jax.lax.top_k);
      counts via match_replace marking; stats accumulated in SBUF
    - final cross-partition stats reduction via a ones-vector matmul
Host: gathers per-core outputs, computes the scalar losses in numpy.
"""
import numpy as np
from contextlib import ExitStack

import concourse.tile as tile
from concourse import bacc, mybir
from concourse.bass_utils import run_bass_kernel_spmd
from concourse.masks import make_identity

F32 = mybir.dt.float32
F16 = mybir.dt.float16
I32 = mybir.dt.int32
U32 = mybir.dt.uint32

T_FULL = 16384
H = 4096
E = 64
N_CORES = 8
TOP_K = 2
T_CORE = T_FULL // N_CORES        # 2048
ST = 512                          # tokens per super-tile
N_ST = T_CORE // ST               # 4
NJ = ST // 128                    # 4
SW = NJ * E
STATS_W = 2 * SW
KCH = H // 128                    # 32
NT = T_CORE // 128                # 16
EPS = 1e-9

Exp = mybir.ActivationFunctionType.Exp
Alu = mybir.AluOpType
AX = mybir.AxisListType


def build_program():
    nc = bacc.Bacc("TRN2", target_bir_lowering=False, debug=False,
                   num_devices=N_CORES)
    x_d = nc.dram_tensor("x", [T_CORE, H], F32, kind="ExternalInput").ap()
    g_d = nc.dram_tensor("g", [128, KCH * 2 * E], F16, kind="ExternalInput").ap()
    w_d = nc.dram_tensor("w", [128, NT, 2], F32, kind="ExternalOutput").ap()
    i_d = nc.dram_tensor("i", [128, NT, 2], I32, kind="ExternalOutput").ap()
    s_d = nc.dram_tensor("s", [1, STATS_W], F32, kind="ExternalOutput").ap()
    z_d = nc.dram_tensor("z", [128, NT, 2], F32, kind="ExternalOutput").ap()

    with tile.TileContext(nc) as tc, ExitStack() as ctx:
        cst = ctx.enter_context(tc.tile_pool(name="cst", bufs=1))
        nat_p = ctx.enter_context(tc.tile_pool(name="nat", bufs=8))
        xT_p = ctx.enter_context(tc.tile_pool(name="xT", bufs=6))
        sm_p = ctx.enter_context(tc.tile_pool(name="sm", bufs=2))
        ps_tr = ctx.enter_context(tc.tile_pool(name="ps_tr", bufs=3, space="PSUM"))
        ps_lg = ctx.enter_context(tc.tile_pool(name="ps_lg", bufs=2, space="PSUM"))
        ps_lt = ctx.enter_context(tc.tile_pool(name="ps_lt", bufs=2, space="PSUM"))
        ps_st = ctx.enter_context(tc.tile_pool(name="ps_st", bufs=1, space="PSUM"))

        ident = cst.tile([128, 128], F32)
        make_identity(nc, ident[:])
        ones = cst.tile([128, 1], F32)
        nc.vector.memset(ones[:], 1.0)
        gt = cst.tile([128, KCH, 2 * E], F16)
        mr = cst.tile([128, 8], F32)
        nc.vector.memset(mr[:], -1.0)
        stats = cst.tile([128, STATS_W], F32)
        nc.vector.memset(stats[:], 0.0)
        w_all = cst.tile([128, NT, 2], F32)
        i_all = cst.tile([128, NT, 2], I32)
        z_all = cst.tile([128, NT, 2], F32)

        for s in range(N_ST):
            nat = []
            if s == 0:
                # chunked, j-interleaved first loads: transposes can start
                # after one chunk-set instead of after all NJ full tiles
                CH = H // 4
                for j in range(NJ):
                    t = nat_p.tile([128, H], F32, tag="nat",
                                   name=f"nat0_{j}")
                    nat.append(t)
                for c in range(4):
                    for j in range(NJ):
                        r0 = j * 128
                        nc.sync.dma_start(
                            nat[j][:, c * CH:(c + 1) * CH],
                            x_d[r0:r0 + 128, c * CH:(c + 1) * CH])
                    if c == 0:
                        nc.sync.dma_start(
                            gt[:], g_d.rearrange("p (k e) -> p k e", k=KCH))
            else:
                for j in range(NJ):
                    t = nat_p.tile([128, H], F32, tag="nat")
                    nc.sync.dma_start(
                        t[:],
                        x_d[(s * NJ + j) * 128:(s * NJ + j + 1) * 128, :])
                    nat.append(t)

            lg = ps_lg.tile([128, ST], F32, tag="lg")
            xTs = [None] * KCH

            def emit_mm(k):
                xv = xTs[k][:].bitcast(F16).rearrange(
                    "p (t two) -> p two t", two=2)
                nc.tensor.matmul(lg[:], gt[:, k, :], xv[:, 0, :],
                                 start=(k == 0), stop=False)
                nc.tensor.matmul(lg[:], gt[:, k, :], xv[:, 1, :],
                                 start=False, stop=(k == KCH - 1))

            for k in range(KCH):
                trp = ps_tr.tile([128, ST], F32, tag="trp")
                for j in range(NJ):
                    nc.tensor.transpose(trp[:, j * 128:(j + 1) * 128],
                                        nat[j][:, k * 128:(k + 1) * 128],
                                        ident[:])
                xT = xT_p.tile([128, ST], F32, tag="xT")
                nc.vector.tensor_copy(xT[:, 0:ST // 2], trp[:, 0:ST // 2])
                nc.scalar.copy(xT[:, ST // 2:ST], trp[:, ST // 2:ST])
                xTs[k] = xT
                # software pipeline: matmul trails transposes by one chunk
                if k >= 1:
                    emit_mm(k - 1)
            emit_mm(KCH - 1)

            lg_sb = sm_p.tile([E, ST], F32, tag="lg_sb")
            nc.scalar.copy(lg_sb[:], lg[0:E, :])
            nc.vector.tensor_add(lg_sb[:], lg_sb[:], lg[E:128, :])
            ltr = ps_lt.tile([128, NJ, E], F32, tag="ltr")
            for j in range(NJ):
                nc.tensor.transpose(ltr[:, j, :],
                                    lg_sb[:, j * 128:(j + 1) * 128],
                                    ident[0:E, 0:E])

            # ---- softmax + top2 + stats on [128, NJ, E] ----
            negmax = sm_p.tile([128, NJ], F32, tag="negmax")
            nc.vector.tensor_reduce(negmax[:], ltr[:], axis=AX.X, op=Alu.max,
                                    negate=True)
            e_st = sm_p.tile([128, NJ, E], F32, tag="e_st")
            zsum = sm_p.tile([128, NJ], F32, tag="zsum")
            for j in range(NJ):
                nc.scalar.activation(e_st[:, j, :], ltr[:, j, :], Exp,
                                     bias=negmax[:, j:j + 1], scale=1.0,
                                     accum_out=zsum[:, j:j + 1])
            rz = sm_p.tile([128, NJ], F32, tag="rz")
            nc.vector.reciprocal(rz[:], zsum[:])
            probs = sm_p.tile([128, NJ, E], F32, tag="probs")
            for j in range(NJ):
                nc.vector.tensor_scalar_mul(probs[:, j, :], e_st[:, j, :],
                                            rz[:, j:j + 1])
            v8 = sm_p.tile([128, NJ, 8], F32, tag="v8")
            i8 = sm_p.tile([128, NJ, 8], U32, tag="i8")
            ind = sm_p.tile([128, NJ, E], F32, tag="ind")
            wsum = sm_p.tile([128, NJ], F32, tag="wsum")
            for j in range(NJ):
                nc.vector.max(v8[:, j, :], probs[:, j, :])
                nc.vector.max_index(i8[:, j, :], v8[:, j, :], probs[:, j, :])
                nc.vector.tensor_copy(mr[:, 0:2], v8[:, j, 0:2])
                marked = sm_p.tile([128, E], F32, tag="marked")
                nc.vector.match_replace(marked[:], mr[:], probs[:, j, :], 3.0)
                nc.vector.tensor_scalar(ind[:, j, :], marked[:], 2.0, None,
                                        op0=Alu.is_ge)
                nc.vector.tensor_scalar(wsum[:, j:j + 1], v8[:, j, 0:1],
                                        v8[:, j, 1:2], EPS, op0=Alu.add,
                                        op1=Alu.add)
            rw = sm_p.tile([128, NJ], F32, tag="rw")
            nc.vector.reciprocal(rw[:], wsum[:])
            for j in range(NJ):
                nc.vector.tensor_scalar_mul(w_all[:, s * NJ + j, :],
                                            v8[:, j, 0:2], rw[:, j:j + 1])
            nc.vector.tensor_copy(i_all[:, s * NJ:(s + 1) * NJ, :],
                                  i8[:, :, 0:2])

            # z-loss finished on host: ship zsum + negmax per token
            nc.vector.tensor_copy(z_all[:, s * NJ:(s + 1) * NJ, 0], zsum[:])
            nc.vector.tensor_copy(z_all[:, s * NJ:(s + 1) * NJ, 1], negmax[:])

            # accumulate per-partition stats
            pr_flat = probs[:].rearrange("p j e -> p (j e)")
            in_flat = ind[:].rearrange("p j e -> p (j e)")
            nc.vector.tensor_add(stats[:, 0:SW], stats[:, 0:SW], pr_flat)
            nc.vector.tensor_add(stats[:, SW:2 * SW], stats[:, SW:2 * SW],
                                 in_flat)

        # cross-partition reduction of stats via ones-matmul
        st_ps = ps_st.tile([1, STATS_W], F32, tag="st_ps")
        for off in range(0, STATS_W, 512):
            hi_off = min(off + 512, STATS_W)
            nc.tensor.matmul(st_ps[:, off:hi_off], ones[:],
                             stats[:, off:hi_off], start=True, stop=True)
        st_sb = cst.tile([1, STATS_W], F32)
        nc.vector.tensor_copy(st_sb[:], st_ps[:])

        nc.sync.dma_start(w_d[:], w_all[:])
        nc.sync.dma_start(i_d[:], i_all[:])
        nc.sync.dma_start(s_d[:], st_sb[:])
        nc.sync.dma_start(z_d[:], z_all[:])

    nc.compile()
    return nc


_NC_CACHE = None


def _get_program():
    global _NC_CACHE
    if _NC_CACHE is None:
        _NC_CACHE = build_program()
    return _NC_CACHE


def pack_gate(gw):
    # gateT chunks: gt[p, k, :] covers hidden rows 128k+p; [hi | lo] planes
    gt_full = gw.T.reshape(KCH, 128, E).transpose(1, 0, 2)
    ghi = gt_full.astype(np.float16)
    glo = (gt_full - ghi.astype(np.float32)).astype(np.float16)
    packed = np.concatenate([ghi, glo], axis=2)
    return np.ascontiguousarray(packed.reshape(128, KCH * 2 * E))


def pack_hidden(hs):
    # fp16 (hi, lo) pair per element, packed into the same 4 bytes
    hi = hs.astype(np.float16)
    lo = (hs - hi.astype(np.float32)).astype(np.float16)
    packed = np.empty((hs.shape[0], hs.shape[1], 2), np.float16)
    packed[:, :, 0] = hi
    packed[:, :, 1] = lo
    return packed.reshape(hs.shape[0], -1).view(np.float32)


def kernel(hidden_states, gate_weight, trace=False):
    hs = np.ascontiguousarray(np.asarray(hidden_states, dtype=np.float32))
    gw = np.ascontiguousarray(np.asarray(gate_weight, dtype=np.float32))
    assert hs.shape == (T_FULL, H) and gw.shape == (E, H)

    nc = _get_program()
    g_packed = pack_gate(gw)
    hs_k = pack_hidden(hs)
    in_maps = [{"x": hs_k[c * T_CORE:(c + 1) * T_CORE], "g": g_packed}
               for c in range(N_CORES)]

    res = None
    last_err = None
    for _attempt in range(3):
        try:
            res = run_bass_kernel_spmd(nc, in_maps, list(range(N_CORES)),
                                       trace=trace)
            # materialize (errors can surface lazily)
            for c in range(N_CORES):
                for k in list(res.results[c]):
                    res.results[c][k] = np.asarray(res.results[c][k])
            break
        except Exception as e:  # flaky device recovery; retry
            last_err = e
            res = None
    if res is None:
        raise last_err
    kernel.last_result = res

    w_parts, i_parts = [], []
    p_sum = np.zeros(E, np.float64)
    counts = np.zeros(E, np.float64)
    zsq_tot = 0.0
    for c in range(N_CORES):
        r = res.results[c]
        w_parts.append(r["w"].transpose(1, 0, 2).reshape(T_CORE, 2))
        i_parts.append(r["i"].transpose(1, 0, 2).reshape(T_CORE, 2))
        st = r["s"][0]
        p_sum += st[0:SW].reshape(NJ, E).sum(0)
        counts += st[SW:2 * SW].reshape(NJ, E).sum(0)
        zz = r["z"].astype(np.float64)
        logz = np.log(zz[:, :, 0]) - zz[:, :, 1]
        zsq_tot += float(np.sum(logz ** 2))

    expert_weights = np.concatenate(w_parts, 0)
    expert_indices = np.concatenate(i_parts, 0).astype(np.int32)

    P = p_sum / T_FULL
    f = counts / (T_FULL * TOP_K)
    lb_loss = np.float32(0.01 * E * np.sum(f * P))
    z_loss = np.float32(0.001 * zsq_tot / T_FULL)
    expert_util = (counts / (T_FULL * TOP_K)).astype(np.float32)
    return (expert_weights, expert_indices, lb_loss, z_loss, expert_util)


kernel.last_result = None


# revision 4
# speedup vs baseline: 1.0455x; 1.0455x over previous
"""Balanced top-2 MoE router on 8 TRN2 NeuronCores (token-parallel).

Reference computation:
    logits = hidden @ gate.T           [T=16384, E=64], H=4096
    probs  = softmax(logits); (w, i) = top2(probs); w /= (w.sum() + 1e-9)
    lb_loss / z_loss / expert_util from counts, P-mean and logsumexp stats.

Sharding: tokens split 8 ways (2048/core), gate replicated (sharding hint).

Host side packs both operands into an fp16 (hi, lo) pair representation
(hi = fp16(x), lo = fp16(x - hi)); the pair occupies the same 4 bytes per
element as the original fp32, so DMA volume is unchanged and the device
kernel reconstructs the full-precision product as
  (ghi + glo)^T @ (xhi + xlo)
with four fp16 matmul terms that run at 4x the fp32 matmul rate.

Per-core device kernel (per 512-token super-tile):
    - DMA hidden slice in natural [token, H] layout (contiguous, full BW)
    - PE transpose-mode matmuls turn [128t, 128h] blocks into [128h, 128t]
    - PSUM -> SBUF copies (split DVE/ACT) assemble xT [128h, 512t] tiles
    - 2 fp16 matmuls per contraction chunk: stationary [ghi|glo] [128h, 128],
      moving = hi/lo planes of xT via stride-2 fp16 APs -> PSUM [128, 512t];
      logits = top half + bottom half
    - PE transpose logits back to [128t, 64e]
    - softmax: ACT Exp with fused per-token bias (-max) + fused row-sum;
      top-2 via DVE InstMax/InstMaxIndex (ties resolve like jax.lax.top_k);
      counts via match_replace marking; stats accumulated in SBUF
    - final cross-partition stats reduction via a ones-vector matmul
Host: gathers per-core outputs, computes the scalar losses in numpy.
"""
import numpy as np
from contextlib import ExitStack

import concourse.tile as tile
from concourse import bacc, mybir
from concourse.bass_utils import run_bass_kernel_spmd
from concourse.masks import make_identity

F32 = mybir.dt.float32
F16 = mybir.dt.float16
I32 = mybir.dt.int32
U32 = mybir.dt.uint32

T_FULL = 16384
H = 4096
E = 64
N_CORES = 8
TOP_K = 2
T_CORE = T_FULL // N_CORES        # 2048
ST = 512                          # tokens per super-tile
N_ST = T_CORE // ST               # 4
NJ = ST // 128                    # 4
SW = NJ * E
STATS_W = 2 * SW
KCH = H // 128                    # 32
NT = T_CORE // 128                # 16
EPS = 1e-9

Exp = mybir.ActivationFunctionType.Exp
Alu = mybir.AluOpType
AX = mybir.AxisListType


def build_program():
    nc = bacc.Bacc("TRN2", target_bir_lowering=False, debug=False,
                   num_devices=N_CORES)
    x_d = nc.dram_tensor("x", [T_CORE, H], F32, kind="ExternalInput").ap()
    g_d = nc.dram_tensor("g", [128, KCH * 2 * E], F16, kind="ExternalInput").ap()
    w_d = nc.dram_tensor("w", [128, NT, 2], F32, kind="ExternalOutput").ap()
    i_d = nc.dram_tensor("i", [128, NT, 2], I32, kind="ExternalOutput").ap()
    s_d = nc.dram_tensor("s", [1, STATS_W], F32, kind="ExternalOutput").ap()
    z_d = nc.dram_tensor("z", [128, NT, 2], F32, kind="ExternalOutput").ap()

    with tile.TileContext(nc) as tc, ExitStack() as ctx:
        cst = ctx.enter_context(tc.tile_pool(name="cst", bufs=1))
        nat_p = ctx.enter_context(tc.tile_pool(name="nat", bufs=8))
        xT_p = ctx.enter_context(tc.tile_pool(name="xT", bufs=6))
        sm_p = ctx.enter_context(tc.tile_pool(name="sm", bufs=2))
        ps_tr = ctx.enter_context(tc.tile_pool(name="ps_tr", bufs=3, space="PSUM"))
        ps_lg = ctx.enter_context(tc.tile_pool(name="ps_lg", bufs=2, space="PSUM"))
        ps_lt = ctx.enter_context(tc.tile_pool(name="ps_lt", bufs=2, space="PSUM"))
        ps_st = ctx.enter_context(tc.tile_pool(name="ps_st", bufs=1, space="PSUM"))

        ident = cst.tile([128, 128], F32)
        make_identity(nc, ident[:])
        ones = cst.tile([128, 1], F32)
        nc.vector.memset(ones[:], 1.0)
        gt = cst.tile([128, KCH, 2 * E], F16)
        mr = cst.tile([128, 8], F32)
        nc.vector.memset(mr[:], -1.0)
        stats = cst.tile([128, STATS_W], F32)
        nc.vector.memset(stats[:], 0.0)
        w_all = cst.tile([128, NT, 2], F32)
        i_all = cst.tile([128, NT, 2], I32)
        z_all = cst.tile([128, NT, 2], F32)

        for s in range(N_ST):
            nat = []
            if s == 0:
                # chunked, j-interleaved first loads: transposes can start
                # after one chunk-set instead of after all NJ full tiles
                CH = H // 4
                for j in range(NJ):
                    t = nat_p.tile([128, H], F32, tag="nat",
                                   name=f"nat0_{j}")
                    nat.append(t)
                for c in range(4):
                    for j in range(NJ):
                        r0 = j * 128
                        nc.sync.dma_start(
                            nat[j][:, c * CH:(c + 1) * CH],
                            x_d[r0:r0 + 128, c * CH:(c + 1) * CH])
                    if c == 0:
                        nc.sync.dma_start(
                            gt[:], g_d.rearrange("p (k e) -> p k e", k=KCH))
            else:
                for j in range(NJ):
                    t = nat_p.tile([128, H], F32, tag="nat")
                    nc.sync.dma_start(
                        t[:],
                        x_d[(s * NJ + j) * 128:(s * NJ + j + 1) * 128, :])
                    nat.append(t)

            lg = ps_lg.tile([128, ST], F32, tag="lg")
            xTs = [None] * KCH

            def emit_mm(k):
                xv = xTs[k][:].bitcast(F16).rearrange(
                    "p (t two) -> p two t", two=2)
                nc.tensor.matmul(lg[:], gt[:, k, :], xv[:, 0, :],
                                 start=(k == 0), stop=False)
                nc.tensor.matmul(lg[:], gt[:, k, :], xv[:, 1, :],
                                 start=False, stop=(k == KCH - 1))

            for k in range(KCH):
                trp = ps_tr.tile([128, ST], F32, tag="trp")
                for j in range(NJ):
                    nc.tensor.transpose(trp[:, j * 128:(j + 1) * 128],
                                        nat[j][:, k * 128:(k + 1) * 128],
                                        ident[:])
                xT = xT_p.tile([128, ST], F32, tag="xT")
                SPL = 5 * ST // 8
                nc.vector.tensor_copy(xT[:, 0:SPL], trp[:, 0:SPL])
                nc.scalar.copy(xT[:, SPL:ST], trp[:, SPL:ST])
                xTs[k] = xT
                # software pipeline: matmuls trail transposes by two chunks
                if k >= 2:
                    emit_mm(k - 2)
            emit_mm(KCH - 2)
            emit_mm(KCH - 1)

            lg_sb = sm_p.tile([E, ST], F32, tag="lg_sb")
            nc.scalar.copy(lg_sb[:], lg[0:E, :])
            nc.vector.tensor_add(lg_sb[:], lg_sb[:], lg[E:128, :])
            ltr = ps_lt.tile([128, NJ, E], F32, tag="ltr")
            for j in range(NJ):
                nc.tensor.transpose(ltr[:, j, :],
                                    lg_sb[:, j * 128:(j + 1) * 128],
                                    ident[0:E, 0:E])

            # ---- softmax + top2 + stats on [128, NJ, E] ----
            negmax = sm_p.tile([128, NJ], F32, tag="negmax")
            nc.vector.tensor_reduce(negmax[:], ltr[:], axis=AX.X, op=Alu.max,
                                    negate=True)
            e_st = sm_p.tile([128, NJ, E], F32, tag="e_st")
            zsum = sm_p.tile([128, NJ], F32, tag="zsum")
            for j in range(NJ):
                nc.scalar.activation(e_st[:, j, :], ltr[:, j, :], Exp,
                                     bias=negmax[:, j:j + 1], scale=1.0,
                                     accum_out=zsum[:, j:j + 1])
            rz = sm_p.tile([128, NJ], F32, tag="rz")
            nc.vector.reciprocal(rz[:], zsum[:])
            probs = sm_p.tile([128, NJ, E], F32, tag="probs")
            for j in range(NJ):
                nc.vector.tensor_scalar_mul(probs[:, j, :], e_st[:, j, :],
                                            rz[:, j:j + 1])
            v8 = sm_p.tile([128, NJ, 8], F32, tag="v8")
            i8 = sm_p.tile([128, NJ, 8], U32, tag="i8")
            ind = sm_p.tile([128, NJ, E], F32, tag="ind")
            wsum = sm_p.tile([128, NJ], F32, tag="wsum")
            for j in range(NJ):
                nc.vector.max(v8[:, j, :], probs[:, j, :])
                nc.vector.max_index(i8[:, j, :], v8[:, j, :], probs[:, j, :])
                nc.vector.tensor_copy(mr[:, 0:2], v8[:, j, 0:2])
                marked = sm_p.tile([128, E], F32, tag="marked")
                nc.vector.match_replace(marked[:], mr[:], probs[:, j, :], 3.0)
                nc.vector.tensor_scalar(ind[:, j, :], marked[:], 2.0, None,
                                        op0=Alu.is_ge)
                nc.vector.tensor_scalar(wsum[:, j:j + 1], v8[:, j, 0:1],
                                        v8[:, j, 1:2], EPS, op0=Alu.add,
                                        op1=Alu.add)
            rw = sm_p.tile([128, NJ], F32, tag="rw")
            nc.vector.reciprocal(rw[:], wsum[:])
            for j in range(NJ):
                nc.vector.tensor_scalar_mul(w_all[:, s * NJ + j, :],
                                            v8[:, j, 0:2], rw[:, j:j + 1])
            nc.vector.tensor_copy(i_all[:, s * NJ:(s + 1) * NJ, :],
                                  i8[:, :, 0:2])

            # z-loss finished on host: ship zsum + negmax per token
            nc.vector.tensor_copy(z_all[:, s * NJ:(s + 1) * NJ, 0], zsum[:])
            nc.vector.tensor_copy(z_all[:, s * NJ:(s + 1) * NJ, 1], negmax[:])

            # accumulate per-partition stats
            pr_flat = probs[:].rearrange("p j e -> p (j e)")
            in_flat = ind[:].rearrange("p j e -> p (j e)")
            nc.vector.tensor_add(stats[:, 0:SW], stats[:, 0:SW], pr_flat)
            nc.vector.tensor_add(stats[:, SW:2 * SW], stats[:, SW:2 * SW],
                                 in_flat)

        # cross-partition reduction of stats via ones-matmul
        st_ps = ps_st.tile([1, STATS_W], F32, tag="st_ps")
        for off in range(0, STATS_W, 512):
            hi_off = min(off + 512, STATS_W)
            nc.tensor.matmul(st_ps[:, off:hi_off], ones[:],
                             stats[:, off:hi_off], start=True, stop=True)
        st_sb = cst.tile([1, STATS_W], F32)
        nc.vector.tensor_copy(st_sb[:], st_ps[:])

        nc.sync.dma_start(w_d[:], w_all[:])
        nc.sync.dma_start(i_d[:], i_all[:])
        nc.sync.dma_start(s_d[:], st_sb[:])
        nc.sync.dma_start(z_d[:], z_all[:])

    nc.compile()
    return nc


_NC_CACHE = None


def _get_program():
    global _NC_CACHE
    if _NC_CACHE is None:
        _NC_CACHE = build_program()
    return _NC_CACHE


def pack_gate(gw):
    # gateT chunks: gt[p, k, :] covers hidden rows 128k+p; [hi | lo] planes
    gt_full = gw.T.reshape(KCH, 128, E).transpose(1, 0, 2)
    ghi = gt_full.astype(np.float16)
    glo = (gt_full - ghi.astype(np.float32)).astype(np.float16)
    packed = np.concatenate([ghi, glo], axis=2)
    return np.ascontiguousarray(packed.reshape(128, KCH * 2 * E))


def pack_hidden(hs):
    # fp16 (hi, lo) pair per element, packed into the same 4 bytes
    hi = hs.astype(np.float16)
    lo = (hs - hi.astype(np.float32)).astype(np.float16)
    packed = np.empty((hs.shape[0], hs.shape[1], 2), np.float16)
    packed[:, :, 0] = hi
    packed[:, :, 1] = lo
    return packed.reshape(hs.shape[0], -1).view(np.float32)


def kernel(hidden_states, gate_weight, trace=False):
    hs = np.ascontiguousarray(np.asarray(hidden_states, dtype=np.float32))
    gw = np.ascontiguousarray(np.asarray(gate_weight, dtype=np.float32))
    assert hs.shape == (T_FULL, H) and gw.shape == (E, H)

    nc = _get_program()
    g_packed = pack_gate(gw)
    hs_k = pack_hidden(hs)
    in_maps = [{"x": hs_k[c * T_CORE:(c + 1) * T_CORE], "g": g_packed}
               for c in range(N_CORES)]

    res = None
    last_err = None
    for _attempt in range(3):
        try:
            res = run_bass_kernel_spmd(nc, in_maps, list(range(N_CORES)),
                                       trace=trace)
            # materialize (errors can surface lazily)
            for c in range(N_CORES):
                for k in list(res.results[c]):
                    res.results[c][k] = np.asarray(res.results[c][k])
            break
        except Exception as e:  # flaky device recovery; retry
            last_err = e
            res = None
    if res is None:
        raise last_err
    kernel.last_result = res

    w_parts, i_parts = [], []
    p_sum = np.zeros(E, np.float64)
    counts = np.zeros(E, np.float64)
    zsq_tot = 0.0
    for c in range(N_CORES):
        r = res.results[c]
        w_parts.append(r["w"].transpose(1, 0, 2).reshape(T_CORE, 2))
        i_parts.append(r["i"].transpose(1, 0, 2).reshape(T_CORE, 2))
        st = r["s"][0]
        p_sum += st[0:SW].reshape(NJ, E).sum(0)
        counts += st[SW:2 * SW].reshape(NJ, E).sum(0)
        zz = r["z"].astype(np.float64)
        logz = np.log(zz[:, :, 0]) - zz[:, :, 1]
        zsq_tot += float(np.sum(logz ** 2))

    expert_weights = np.concatenate(w_parts, 0)
    expert_indices = np.concatenate(i_parts, 0).astype(np.int32)

    P = p_sum / T_FULL
    f = counts / (T_FULL * TOP_K)
    lb_loss = np.float32(0.01 * E * np.sum(f * P))
    z_loss = np.float32(0.001 * zsq_tot / T_FULL)
    expert_util = (counts / (T_FULL * TOP_K)).astype(np.float32)
    return (expert_weights, expert_indices, lb_loss, z_loss, expert_util)


kernel.last_result = None
